# revision 1
# baseline (speedup 1.0000x reference)
"""LocalWindowAttention Trainium2 kernel.

Problem: B=8, S=4096, D=1024, H=16 heads, hd=64, window W=64.
  qkv = x @ qkv_w + qkv_b; per-window attention with relative position
  bias; out = attn_out @ proj_w + proj_b.

Sharding: data-parallel over batch — one batch element per NeuronCore
(8 cores), no collectives needed.

Per-core pipeline (S=4096 rows, processed in s-tiles of 512 rows):
  1. x row-blocks [128,1024] are PE-transposed into xT (feature-major).
  2. qT/kT (feature-major) and v (seq-major) via fp16 matmuls vs resident
     qkv_w tiles; fp32 PSUM accumulation over the K=1024 contraction.
  3. Attention per 128-row block (= 2 windows of 64, masked to be
     independent via -1e4 in the bias tile) and per group of 4 heads:
       scoresT[k,q] = kT.T @ qT   (transposed scores — avoids per-head
                                   transposes downstream)
       attnT = exp(scoresT + biasT)       (no max-subtraction: scores are
                                           O(+-6) by construction)
       outT_unnorm[q,hd], denom[q] = attnT.T @ [v | 1]  (ones column fused
                                                         into v gives the
                                                         softmax denominator)
       attn_out[q,hd] = outT_unnorm * (1/denom)  (per-partition scale)
  4. attn_out PE-transposed per 128-col block; proj matmul; DMA out.

Scale 1/sqrt(hd) is folded into qkv_w's q-columns host-side. qkv_b's
v-part is folded into an effective proj bias host-side (rows of attn sum
to 1). All matmul operands are fp16 (error ~5e-4 vs fp32 reference);
accumulation is always fp32.
"""
import os
import numpy as np

import concourse.bacc as bacc
import concourse.mybir as mybir
from concourse.tile import TileContext
from concourse.bass_utils import run_bass_kernel_spmd
from concourse.masks import make_identity

F16 = mybir.dt.float16
F32 = mybir.dt.float32

B, S, D = 8, 4096, 1024
H, W, HD = 16, 64, 64
NW = S // W              # 64 windows
STILE = 512              # seq rows per pipeline tile
NST = S // STILE         # 8 s-tiles
NBLK = STILE // 128      # 4 row-blocks (window pairs) per s-tile
MASK = -10000.0          # exp() underflows to exactly 0 in fp16/fp32


def _build(n_stiles=NST, with_qkbias=False, with_projbias=False):
    nc = bacc.Bacc()
    s_total = n_stiles * STILE

    x_ext = nc.declare_dram_parameter("x16", [s_total, D], F16, isOutput=False)
    w_ext = nc.declare_dram_parameter("qkvw16", [D, 3 * D], F16, isOutput=False)
    pw_ext = nc.declare_dram_parameter("projw16", [D, D], F16, isOutput=False)
    b2_ext = nc.declare_dram_parameter("bias2t16", [128, H * 128], F16,
                                       isOutput=False)
    out_ext = nc.declare_dram_parameter("out", [s_total, D], F32, isOutput=True)
    if with_qkbias:
        qkb_ext = nc.declare_dram_parameter("qkb", [16, 128, 1], F32,
                                            isOutput=False)
    if with_projbias:
        pbb_ext = nc.declare_dram_parameter("projb_bcast", [128, D], F32,
                                            isOutput=False)

    with TileContext(nc) as tc:
        with (
            tc.tile_pool(name="const", bufs=1) as const,
            tc.tile_pool(name="xp", bufs=3) as xp,
            tc.tile_pool(name="xtp", bufs=2) as xtp,
            tc.tile_pool(name="qktp", bufs=2) as qktp,
            tc.tile_pool(name="vap", bufs=6) as vap,
            tc.tile_pool(name="scbp", bufs=3) as scbp,
            tc.tile_pool(name="attp", bufs=3) as attp,
            tc.tile_pool(name="rcp", bufs=4) as rcp,
            tc.tile_pool(name="aout", bufs=2) as aout,
            tc.tile_pool(name="atp", bufs=2) as atp,
            tc.tile_pool(name="outp", bufs=2) as outp,
            tc.tile_pool(name="acc", bufs=2, space="PSUM") as acc,
            tc.tile_pool(name="tp", bufs=2, space="PSUM") as tp,
            tc.tile_pool(name="scps", bufs=2, space="PSUM") as scps,
            tc.tile_pool(name="aops", bufs=2, space="PSUM") as aops,
        ):
            # ---- resident constants -------------------------------------
            wts = []
            for k in range(8):
                wk = const.tile([128, 3 * D], F16, name=f"wk{k}")
                nc.sync.dma_start(out=wk[:], in_=w_ext[k * 128:(k + 1) * 128, :])
                wts.append(wk)
            pwts = []
            for k in range(8):
                pk = const.tile([128, D], F16, name=f"pk{k}")
                nc.sync.dma_start(out=pk[:], in_=pw_ext[k * 128:(k + 1) * 128, :])
                pwts.append(pk)
            b2t = const.tile([128, H * 128], F16, name="b2t")
            nc.sync.dma_start(out=b2t[:], in_=b2_ext[:])
            ident = const.tile([128, 128], F16, name="ident")
            make_identity(nc, ident)
            if with_qkbias:
                qkb = const.tile([128, 16], F32, name="qkb")
                for m in range(16):
                    nc.sync.dma_start(out=qkb[:, m:m + 1], in_=qkb_ext[m])
            if with_projbias:
                pbb = const.tile([128, D], F32, name="pbb")
                nc.sync.dma_start(out=pbb[:], in_=pbb_ext[:])

            # ---- main loop over s-tiles ---------------------------------
            for st in range(n_stiles):
                s0 = st * STILE

                # stage 1: load x, transpose to feature-major xT
                xt = xtp.tile([128, 8 * STILE], F16, name="xt")
                for b in range(NBLK):
                    xb = xp.tile([128, D], F16, name="xb")
                    nc.sync.dma_start(
                        out=xb[:], in_=x_ext[s0 + b * 128:s0 + (b + 1) * 128, :])
                    for c in range(8):
                        tpp = tp.tile([128, 128], F16, name="tpp")
                        nc.tensor.transpose(
                            tpp[:], xb[:, c * 128:(c + 1) * 128], ident[:])
                        nc.vector.tensor_copy(
                            xt[:, c * STILE + b * 128:c * STILE + b * 128 + 128],
                            tpp[:])

                # stage 2a: qT, kT (feature-major, fp16)
                qt = qktp.tile([128, 8 * STILE], F16, name="qt", tag="qt")
                kt = qktp.tile([128, 8 * STILE], F16, name="kt", tag="kt")
                for which, dst in ((0, qt), (1, kt)):
                    for m in range(8):
                        ac = acc.tile([128, STILE], F32, name="ac")
                        col0 = which * D + m * 128
                        for k in range(8):
                            nc.tensor.matmul(
                                ac[:],
                                wts[k][:, col0:col0 + 128],
                                xt[:, k * STILE:(k + 1) * STILE],
                                start=(k == 0), stop=(k == 7))
                        dsl = dst[:, m * STILE:(m + 1) * STILE]
                        if with_qkbias:
                            nc.scalar.activation(
                                dsl, ac[:], mybir.ActivationFunctionType.Identity,
                                bias=qkb[:, which * 8 + m:which * 8 + m + 1])
                        else:
                            nc.scalar.copy(dsl, ac[:])

                # stage 2b: v (seq-major, ones column appended per head)
                vts = []
                for b in range(NBLK):
                    vt = vap.tile([128, H * (HD + 1)], F16, name="vt")
                    vt3 = vt.rearrange("p (h c) -> p h c", c=HD + 1)
                    nc.vector.memset(vt3[:, :, HD:HD + 1], 1.0)
                    for n in range(2):
                        ac = acc.tile([128, STILE], F32, name="ac")
                        for k in range(8):
                            nc.tensor.matmul(
                                ac[:],
                                xt[:, k * STILE + b * 128:k * STILE + b * 128 + 128],
                                wts[k][:, 2 * D + n * 512:2 * D + (n + 1) * 512],
                                start=(k == 0), stop=(k == 7))
                        nc.vector.tensor_copy(
                            vt3[:, n * 8:(n + 1) * 8, 0:HD],
                            ac.rearrange("p (h c) -> p h c", c=HD))
                    vts.append(vt)

                # stage 3: attention per window-pair block, 4 heads at a time
                for p in range(NBLK):
                    ao = aout.tile([128, D], F16, name="ao")
                    for hg in range(4):
                        # one PSUM tile per matmul group: sharing a bank
                        # between independent PE write-groups and DVE
                        # readers crashes the hardware
                        scb = scbp.tile([128, 512], F32, name="scb")
                        for i in range(4):
                            h = hg * 4 + i
                            m, r = h // 2, (h % 2) * 64
                            c0 = m * STILE + p * 128
                            sc = scps.tile([128, 128], F32, name="sc")
                            nc.tensor.matmul(
                                sc[:],
                                kt[r:r + 64, c0:c0 + 128],
                                qt[r:r + 64, c0:c0 + 128],
                                start=True, stop=True)
                            nc.vector.tensor_add(
                                scb[:, i * 128:(i + 1) * 128], sc[:],
                                b2t[:, h * 128:(h + 1) * 128])
                        att = attp.tile([128, 512], F16, name="att")
                        nc.scalar.activation(
                            att[:], scb[:], mybir.ActivationFunctionType.Exp)
                        vt3 = vts[p].rearrange("p (h c) -> p h c", c=HD + 1)
                        for i in range(4):
                            h = hg * 4 + i
                            aop = aops.tile([128, 128], F32, name="aop")
                            nc.tensor.matmul(
                                aop[:, :65],
                                att[:, i * 128:(i + 1) * 128],
                                vt3[:, h, :],
                                start=True, stop=True)
                            rc = rcp.tile([128, 1], F32, name="rc")
                            nc.vector.reciprocal(rc[:], aop[:, 64:65])
                            nc.vector.tensor_scalar_mul(
                                ao[:, h * 64:(h + 1) * 64],
                                aop[:, :64],
                                rc[:])

                    # stage 4: transpose attn_out, proj matmul, store
                    at = atp.tile([128, D], F16, name="at")
                    for c in range(8):
                        tpp = tp.tile([128, 128], F16, name="tpp")
                        nc.tensor.transpose(
                            tpp[:], ao[:, c * 128:(c + 1) * 128], ident[:])
                        nc.vector.tensor_copy(at[:, c * 128:(c + 1) * 128], tpp[:])
                    ot = outp.tile([128, D], F32, name="ot")
                    for n in range(2):
                        ac = acc.tile([128, STILE], F32, name="ac")
                        for k in range(8):
                            nc.tensor.matmul(
                                ac[:],
                                at[:, k * 128:(k + 1) * 128],
                                pwts[k][:, n * 512:(n + 1) * 512],
                                start=(k == 0), stop=(k == 7))
                        nc.scalar.copy(ot[:, n * 512:(n + 1) * 512], ac[:])
                    if with_projbias:
                        nc.vector.tensor_add(ot[:], ot[:], pbb[:])
                    nc.sync.dma_start(
                        out=out_ext[s0 + p * 128:s0 + (p + 1) * 128, :],
                        in_=ot[:])

    nc.compile()
    return nc


def _host_prep(x, qkv_w, qkv_b, proj_w, proj_b, rel_bias):
    """Fold scale/biases, cast to fp16, build the blocked bias table."""
    scale = 1.0 / np.sqrt(HD)
    qkv_w_s = np.asarray(qkv_w, dtype=np.float64).copy()
    qkv_w_s[:, :D] *= scale
    qkv_b = np.asarray(qkv_b, dtype=np.float64)
    qkv_b_s = qkv_b.copy()
    qkv_b_s[:D] *= scale

    # rel-bias expanded to [H, W, W] then packed into the transposed,
    # window-pair-masked [128 (k), H*128 (h-major, q)] table.
    rb = np.asarray(rel_bias, dtype=np.float32)
    coords = np.arange(W)
    rel = coords[:, None] - coords[None, :] + (W - 1)      # [q, k]
    bias_hqk = rb[rel].transpose(2, 0, 1)                  # [H, q, k]
    b2 = np.full((H, 128, 128), MASK, dtype=np.float32)    # [H, k2, q2]
    bias_kq = bias_hqk.transpose(0, 2, 1)                  # [H, k, q]
    b2[:, :64, :64] = bias_kq
    b2[:, 64:, 64:] = bias_kq
    bias2t16 = np.ascontiguousarray(
        b2.transpose(1, 0, 2).reshape(128, H * 128)).astype(np.float16)

    # v-bias commutes through attention (rows sum to 1) -> fold into proj_b
    proj_b_eff = (qkv_b[2 * D:] @ np.asarray(proj_w, dtype=np.float64)
                  + np.asarray(proj_b, dtype=np.float64))

    shared = {
        "qkvw16": qkv_w_s.astype(np.float16),
        "projw16": np.asarray(proj_w).astype(np.float16),
        "bias2t16": bias2t16,
    }
    qk_bias = qkv_b_s[:2 * D]
    with_qkbias = bool(np.any(qk_bias))
    if with_qkbias:
        shared["qkb"] = np.ascontiguousarray(
            qk_bias.reshape(16, 128, 1).astype(np.float32))
    with_projbias = bool(np.any(proj_b_eff))
    if with_projbias:
        shared["projb_bcast"] = np.broadcast_to(
            proj_b_eff.astype(np.float32), (128, D)).copy()
    return shared, with_qkbias, with_projbias


_NC_CACHE = {}


def kernel(x, qkv_w, qkv_b, proj_w, proj_b, rel_bias):
    x = np.asarray(x)
    shared, wqk, wpb = _host_prep(x, qkv_w, qkv_b, proj_w, proj_b, rel_bias)

    key = (wqk, wpb)
    if key not in _NC_CACHE:
        _NC_CACHE[key] = _build(NST, wqk, wpb)
    nc = _NC_CACHE[key]

    x16 = np.ascontiguousarray(x.astype(np.float16))       # [B, S, D]
    in_maps = [dict(shared, x16=x16[i]) for i in range(B)]
    res = run_bass_kernel_spmd(nc, in_maps, list(range(B)))
    return np.stack([res.results[i]["out"] for i in range(B)], axis=0)


if __name__ == "__main__":
    rng = np.random.default_rng(0)
    x = rng.standard_normal((B, S, D), dtype=np.float32)
    qkv_w = rng.standard_normal((D, 3 * D), dtype=np.float32) / np.sqrt(D)
    proj_w = rng.standard_normal((D, D), dtype=np.float32) / np.sqrt(D)
    out = kernel(x, qkv_w, np.zeros(3 * D, np.float32), proj_w,
                 np.zeros(D, np.float32),
                 rng.standard_normal((2 * W - 1, H), dtype=np.float32) * 0.02)
    print(out.shape, out.dtype)



# revision 7
# speedup vs baseline: 1.1636x; 1.1636x over previous
"""LocalWindowAttention Trainium2 kernel.

Problem: B=8, S=4096, D=1024, H=16 heads, hd=64, window W=64.
  qkv = x @ qkv_w + qkv_b; per-window attention with relative position
  bias; out = attn_out @ proj_w + proj_b.

Sharding: data-parallel over batch — one batch element per NeuronCore
(8 cores), no collectives needed.

Per-core pipeline (S=4096 rows, processed in s-tiles of 512 rows):
  1. x is pre-transposed HOST-side to xT [D, S]; feature-major xt tiles
     stream in with plain wide DMAs (no on-chip transposes).
  2. qT/kT (feature-major) and v (seq-major) via fp16 matmuls vs resident
     qkv_w tiles; fp32 PSUM accumulation over the K=1024 contraction.
  3. Attention per 128-row block (= 2 windows of 64) and per group of 4
     heads:
       scoresT[k,q] = kT.T @ qT      (4 heads -> 4 col-quarters of ONE
                                      [128,512] PSUM tile; the single
                                      reader comes after all 4 writers,
                                      so no PE-write/DVE-read bank race)
       att = exp(scoresT) * expb     (expb = exp(rel_bias) table, fp16;
                                      cross-window entries are exactly 0,
                                      replacing the -1e4 additive mask)
       outT_unnorm[q,hd], denom[q] = att.T @ [v | 1]   (4 heads -> 4
                                      65-col slots of ONE PSUM tile)
       attn_out[q,hd] = outT_unnorm * (1/denom)  (one strided reciprocal
                                      + one broadcast multiply per group)
  4. attn_out PE-transposed per 128-col block; proj matmul; DMA out.

Scale 1/sqrt(hd) is folded into qkv_w's q-columns host-side. qkv_b's
v-part is folded into an effective proj bias host-side (rows of attn sum
to 1). All matmul operands are fp16 (error ~1e-3 vs fp32 reference);
accumulation is always fp32.
"""
import os
import numpy as np

import concourse.bacc as bacc
import concourse.mybir as mybir
from concourse.tile import TileContext
from concourse.bass_utils import run_bass_kernel_spmd
from concourse.masks import make_identity

F16 = mybir.dt.float16
F32 = mybir.dt.float32

B, S, D = 8, 4096, 1024
H, W, HD = 16, 64, 64
NW = S // W              # 64 windows
STILE = 512              # seq rows per pipeline tile
NST = S // STILE         # 8 s-tiles
NBLK = STILE // 128      # 4 row-blocks (window pairs) per s-tile
MASK = -10000.0          # exp() underflows to exactly 0


# Feature switches (env-overridable for testing). SC_SHARED=1 (4 score
# matmuls into col-quarters of one PSUM tile) CRASHES the hardware —
# apparently the base-partition-64 stationary (odd heads) combined with a
# non-zero PSUM byte offset; the av pattern (all base partition 0) is fine.
SC_SHARED = os.environ.get("KSC", "0") == "1"   # 4 score mms -> one PSUM tile
AV_SHARED = os.environ.get("KAV", "1") == "1"   # 4 av mms -> one PSUM tile
BCAST_NORM = os.environ.get("KBC", "1") == "1"  # batched bcast normalize


def _build(n_stiles=NST, with_qkbias=False, with_projbias=False):
    nc = bacc.Bacc()
    s_total = n_stiles * STILE

    xt_ext = nc.declare_dram_parameter("xt16", [D, s_total], F16, isOutput=False)
    w_ext = nc.declare_dram_parameter("qkvw16", [D, 3 * D], F16, isOutput=False)
    pw_ext = nc.declare_dram_parameter("projw16", [D, D], F16, isOutput=False)
    eb_ext = nc.declare_dram_parameter("expb16", [128, H * 128], F16,
                                       isOutput=False)
    out_ext = nc.declare_dram_parameter("out", [s_total, D], F32, isOutput=True)
    if with_qkbias:
        qkb_ext = nc.declare_dram_parameter("qkb", [16, 128, 1], F32,
                                            isOutput=False)
    if with_projbias:
        pbb_ext = nc.declare_dram_parameter("projb_bcast", [128, D], F32,
                                            isOutput=False)

    with TileContext(nc) as tc:
        with (
            tc.tile_pool(name="const", bufs=1) as const,
            tc.tile_pool(name="xtp", bufs=2) as xtp,
            tc.tile_pool(name="qktp", bufs=2) as qktp,
            tc.tile_pool(name="vap", bufs=8) as vap,
            tc.tile_pool(name="arp", bufs=4) as arp,
            tc.tile_pool(name="attp", bufs=4) as attp,
            tc.tile_pool(name="rcp", bufs=4) as rcp,
            tc.tile_pool(name="aout", bufs=2) as aout,
            tc.tile_pool(name="atp", bufs=2) as atp,
            tc.tile_pool(name="outp", bufs=2) as outp,
            tc.tile_pool(name="acc", bufs=2, space="PSUM") as acc,
            tc.tile_pool(name="scps", bufs=2, space="PSUM") as scps,
            tc.tile_pool(name="aops", bufs=2, space="PSUM") as aops,
            tc.tile_pool(name="tp", bufs=2, space="PSUM") as tp,
        ):
            # ---- resident constants -------------------------------------
            wts = []
            for k in range(8):
                wk = const.tile([128, 3 * D], F16, name=f"wk{k}")
                nc.sync.dma_start(out=wk[:], in_=w_ext[k * 128:(k + 1) * 128, :])
                wts.append(wk)
            pwts = []
            for k in range(8):
                pk = const.tile([128, D], F16, name=f"pk{k}")
                nc.sync.dma_start(out=pk[:], in_=pw_ext[k * 128:(k + 1) * 128, :])
                pwts.append(pk)
            ebt = const.tile([128, H * 128], F16, name="ebt")
            nc.sync.dma_start(out=ebt[:], in_=eb_ext[:])
            ident = const.tile([128, 128], F16, name="ident")
            make_identity(nc, ident)
            if with_qkbias:
                qkb = const.tile([128, 16], F32, name="qkb")
                for m in range(16):
                    nc.sync.dma_start(out=qkb[:, m:m + 1], in_=qkb_ext[m])
            if with_projbias:
                pbb = const.tile([128, D], F32, name="pbb")
                nc.sync.dma_start(out=pbb[:], in_=pbb_ext[:])

            # ---- main loop over s-tiles ---------------------------------
            for st in range(n_stiles):
                s0 = st * STILE

                # stage 1: feature-major xt, straight from the
                # pre-transposed DRAM copy
                xt = xtp.tile([128, 8 * STILE], F16, name="xt")
                for c in range(8):
                    nc.sync.dma_start(
                        out=xt[:, c * STILE:(c + 1) * STILE],
                        in_=xt_ext[c * 128:(c + 1) * 128, s0:s0 + STILE])

                # stage 2a: qT, kT (feature-major, fp16)
                qt = qktp.tile([128, 8 * STILE], F16, name="qt", tag="qt")
                kt = qktp.tile([128, 8 * STILE], F16, name="kt", tag="kt")
                for which, dst in ((0, qt), (1, kt)):
                    for m in range(8):
                        ac = acc.tile([128, STILE], F32, name="ac")
                        col0 = which * D + m * 128
                        for k in range(8):
                            nc.tensor.matmul(
                                ac[:],
                                wts[k][:, col0:col0 + 128],
                                xt[:, k * STILE:(k + 1) * STILE],
                                start=(k == 0), stop=(k == 7))
                        dsl = dst[:, m * STILE:(m + 1) * STILE]
                        if with_qkbias:
                            nc.scalar.activation(
                                dsl, ac[:], mybir.ActivationFunctionType.Identity,
                                bias=qkb[:, which * 8 + m:which * 8 + m + 1])
                        else:
                            nc.scalar.copy(dsl, ac[:])

                # stage 2b: v (seq-major, ones column appended per head)
                vts = []
                for b in range(NBLK):
                    vt = vap.tile([128, H * (HD + 1)], F16, name="vt")
                    vt3 = vt.rearrange("p (h c) -> p h c", c=HD + 1)
                    nc.vector.memset(vt3[:, :, HD:HD + 1], 1.0)
                    for n in range(2):
                        ac = acc.tile([128, STILE], F32, name="ac")
                        for k in range(8):
                            nc.tensor.matmul(
                                ac[:],
                                xt[:, k * STILE + b * 128:k * STILE + b * 128 + 128],
                                wts[k][:, 2 * D + n * 512:2 * D + (n + 1) * 512],
                                start=(k == 0), stop=(k == 7))
                        nc.vector.tensor_copy(
                            vt3[:, n * 8:(n + 1) * 8, 0:HD],
                            ac.rearrange("p (h c) -> p h c", c=HD))
                    vts.append(vt)

                # stage 3: attention per window-pair block, 4 heads at a time
                for p in range(NBLK):
                    ao = aout.tile([128, D], F16, name="ao")
                    vt3 = vts[p].rearrange("p (h c) -> p h c", c=HD + 1)
                    for hg in range(4):
                        araw = arp.tile([128, 512], F16, name="araw")
                        if SC_SHARED:
                            # 4 score matmuls -> col-quarters of one PSUM
                            # tile; the only reader (exp) runs after all
                            # four, so no PE-write/DVE-read bank overlap
                            scb = scps.tile([128, 512], F32, name="scb")
                            for i in range(4):
                                h = hg * 4 + i
                                m, r = h // 2, (h % 2) * 64
                                c0 = m * STILE + p * 128
                                nc.tensor.matmul(
                                    scb[:, i * 128:(i + 1) * 128],
                                    kt[r:r + 64, c0:c0 + 128],
                                    qt[r:r + 64, c0:c0 + 128],
                                    start=True, stop=True)
                            nc.scalar.activation(
                                araw[:], scb[:],
                                mybir.ActivationFunctionType.Exp)
                        else:
                            for i in range(4):
                                h = hg * 4 + i
                                m, r = h // 2, (h % 2) * 64
                                c0 = m * STILE + p * 128
                                sc = scps.tile([128, 128], F32, name="scb")
                                nc.tensor.matmul(
                                    sc[:],
                                    kt[r:r + 64, c0:c0 + 128],
                                    qt[r:r + 64, c0:c0 + 128],
                                    start=True, stop=True)
                                nc.scalar.activation(
                                    araw[:, i * 128:(i + 1) * 128], sc[:],
                                    mybir.ActivationFunctionType.Exp)
                        att = attp.tile([128, 512], F16, name="att")
                        nc.vector.tensor_mul(
                            att[:], araw[:],
                            ebt[:, hg * 512:(hg + 1) * 512])
                        if AV_SHARED:
                            # 4 attention-weighted-V matmuls -> 65-col slots
                            # of one PSUM tile (col 64 of each = denominator)
                            aop4 = aops.tile([128, 4 * (HD + 1)], F32,
                                             name="aop4")
                            a3 = aop4.rearrange("p (i c) -> p i c", c=HD + 1)
                            for i in range(4):
                                h = hg * 4 + i
                                nc.tensor.matmul(
                                    aop4[:, i * (HD + 1):(i + 1) * (HD + 1)],
                                    att[:, i * 128:(i + 1) * 128],
                                    vt3[:, h, :],
                                    start=True, stop=True)
                            if BCAST_NORM:
                                rc4 = rcp.tile([128, 4], F32, name="rc4")
                                nc.vector.reciprocal(rc4[:], a3[:, :, HD])
                                ao3 = ao[:, hg * 256:(hg + 1) * 256].rearrange(
                                    "p (i c) -> p i c", c=HD)
                                nc.vector.tensor_mul(
                                    ao3, a3[:, :, 0:HD],
                                    rc4.rearrange(
                                        "p (i j) -> p i j", j=1).broadcast_to(
                                        [128, 4, HD]))
                            else:
                                for i in range(4):
                                    h = hg * 4 + i
                                    rc = rcp.tile([128, 1], F32, name="rc4")
                                    nc.vector.reciprocal(
                                        rc[:], a3[:, i, HD:HD + 1])
                                    nc.vector.tensor_scalar_mul(
                                        ao[:, h * 64:(h + 1) * 64],
                                        a3[:, i, 0:HD], rc[:])
                        else:
                            for i in range(4):
                                h = hg * 4 + i
                                aop = aops.tile([128, 128], F32, name="aop4")
                                nc.tensor.matmul(
                                    aop[:, :HD + 1],
                                    att[:, i * 128:(i + 1) * 128],
                                    vt3[:, h, :],
                                    start=True, stop=True)
                                rc = rcp.tile([128, 1], F32, name="rc4")
                                nc.vector.reciprocal(rc[:], aop[:, HD:HD + 1])
                                nc.vector.tensor_scalar_mul(
                                    ao[:, h * 64:(h + 1) * 64],
                                    aop[:, :HD], rc[:])

                    # stage 4: transpose attn_out, proj matmul, store
                    at = atp.tile([128, D], F16, name="at")
                    for c in range(8):
                        tpp = tp.tile([128, 128], F16, name="tpp")
                        nc.tensor.transpose(
                            tpp[:], ao[:, c * 128:(c + 1) * 128], ident[:])
                        nc.vector.tensor_copy(at[:, c * 128:(c + 1) * 128], tpp[:])
                    ot = outp.tile([128, D], F32, name="ot")
                    for n in range(2):
                        ac = acc.tile([128, STILE], F32, name="ac")
                        for k in range(8):
                            nc.tensor.matmul(
                                ac[:],
                                at[:, k * 128:(k + 1) * 128],
                                pwts[k][:, n * 512:(n + 1) * 512],
                                start=(k == 0), stop=(k == 7))
                        nc.scalar.copy(ot[:, n * 512:(n + 1) * 512], ac[:])
                    if with_projbias:
                        nc.vector.tensor_add(ot[:], ot[:], pbb[:])
                    nc.sync.dma_start(
                        out=out_ext[s0 + p * 128:s0 + (p + 1) * 128, :],
                        in_=ot[:])

    nc.compile()
    return nc


def _host_prep(x, qkv_w, qkv_b, proj_w, proj_b, rel_bias):
    """Fold scale/biases, cast to fp16, build the exp'd blocked bias table."""
    scale = 1.0 / np.sqrt(HD)
    qkv_w_s = np.asarray(qkv_w, dtype=np.float64).copy()
    qkv_w_s[:, :D] *= scale
    qkv_b = np.asarray(qkv_b, dtype=np.float64)
    qkv_b_s = qkv_b.copy()
    qkv_b_s[:D] *= scale

    # rel-bias expanded to [H, W, W], packed into the transposed,
    # window-pair [128 (k), H*128 (h-major, q)] table, then EXP'd:
    # att = exp(scores) * exp(bias); masked cross-window entries
    # become exactly 0.
    rb = np.asarray(rel_bias, dtype=np.float32)
    coords = np.arange(W)
    rel = coords[:, None] - coords[None, :] + (W - 1)      # [q, k]
    bias_hqk = rb[rel].transpose(2, 0, 1)                  # [H, q, k]
    b2 = np.full((H, 128, 128), MASK, dtype=np.float32)    # [H, k2, q2]
    bias_kq = bias_hqk.transpose(0, 2, 1)                  # [H, k, q]
    b2[:, :64, :64] = bias_kq
    b2[:, 64:, 64:] = bias_kq
    expb16 = np.ascontiguousarray(
        np.exp(b2.transpose(1, 0, 2)).reshape(128, H * 128)).astype(np.float16)

    # v-bias commutes through attention (rows sum to 1) -> fold into proj_b
    proj_b_eff = (qkv_b[2 * D:] @ np.asarray(proj_w, dtype=np.float64)
                  + np.asarray(proj_b, dtype=np.float64))

    shared = {
        "qkvw16": qkv_w_s.astype(np.float16),
        "projw16": np.asarray(proj_w).astype(np.float16),
        "expb16": expb16,
    }
    qk_bias = qkv_b_s[:2 * D]
    with_qkbias = bool(np.any(qk_bias))
    if with_qkbias:
        shared["qkb"] = np.ascontiguousarray(
            qk_bias.reshape(16, 128, 1).astype(np.float32))
    with_projbias = bool(np.any(proj_b_eff))
    if with_projbias:
        shared["projb_bcast"] = np.broadcast_to(
            proj_b_eff.astype(np.float32), (128, D)).copy()
    return shared, with_qkbias, with_projbias


_NC_CACHE = {}


def kernel(x, qkv_w, qkv_b, proj_w, proj_b, rel_bias):
    x = np.asarray(x)
    shared, wqk, wpb = _host_prep(x, qkv_w, qkv_b, proj_w, proj_b, rel_bias)

    key = (wqk, wpb)
    if key not in _NC_CACHE:
        _NC_CACHE[key] = _build(NST, wqk, wpb)
    nc = _NC_CACHE[key]

    # feature-major xT per batch element (seq stays the fast axis on chip)
    xt16 = np.ascontiguousarray(
        x.astype(np.float16).transpose(0, 2, 1))          # [B, D, S]
    in_maps = [dict(shared, xt16=xt16[i]) for i in range(B)]
    res = run_bass_kernel_spmd(nc, in_maps, list(range(B)))
    return np.stack([res.results[i]["out"] for i in range(B)], axis=0)


if __name__ == "__main__":
    rng = np.random.default_rng(0)
    x = rng.standard_normal((B, S, D), dtype=np.float32)
    qkv_w = rng.standard_normal((D, 3 * D), dtype=np.float32) / np.sqrt(D)
    proj_w = rng.standard_normal((D, D), dtype=np.float32) / np.sqrt(D)
    out = kernel(x, qkv_w, np.zeros(3 * D, np.float32), proj_w,
                 np.zeros(D, np.float32),
                 rng.standard_normal((2 * W - 1, H), dtype=np.float32) * 0.02)
    print(out.shape, out.dtype)
